# revision 2
# baseline (speedup 1.0000x reference)
"""BiGCN (bidirectional 2-layer GCN over many small graphs) on 8 Trainium2 cores.

Data-parallel over graphs; 32 graphs x 128 nodes per core.

The GCN norms (D^-1/2 A D^-1/2, self-loops included) depend only on the edge
lists, so the normalized adjacency An^T for every (graph, branch) is built on
the HOST as a dense bf16 [128, 128] block and DMA'd in, along with its row
sums dt = An @ 1.  On-device work is a pure dense-matmul chain with
alternating layouts so no PE transposes are needed:

  y   = X @ [W1_td | W1_bu]                  N-layout [node, 512]     (PE)
  hT  = relu((An @ y)^T) per branch/chunk    H-layout [h, node]       (PE+Act)
  Z   = h @ W2h                              N-layout [node, 256]     (PE)
  H2T = relu((An @ Z)^T + rvec (x) dt)       H-layout [h, node]       (PE+Act)
  out = [mean_d H2, h[:, root]]              (reduce on DVE; transposed once)

rvec = X[roots] @ [W2r_td | W2r_bu] is computed once per core; the root-input
term of layer 2 folds into agg2 as the rank-1 update rvec (x) dt (K=1 matmul).

The per-graph loop is software-pipelined -- iteration i emits
Y(i) | agg1(i-1) | Z(i-2) | agg2(i-3), with Y's six 512-wide matmuls
interleaved between the short agg matmuls so LDWEIGHTS stays hidden and the
PE never stalls (keeping it at the 2.4GHz p-state).
"""

import numpy as np

import concourse.bass as bass
import concourse.tile as tile
from concourse import bacc, mybir
from concourse.bass_utils import run_bass_kernel_spmd
from concourse.masks import make_identity

# Problem shape (fixed by the task)
N_GRAPHS = 256
N_PER_G = 128
IN_FEATS = 768
H_FEATS = 256
N_CORES = 8
G_PER_CORE = N_GRAPHS // N_CORES            # 32
NODES_PER_CORE = G_PER_CORE * N_PER_G       # 4096
KCH = IN_FEATS // 128                       # 6 feature chunks

BF16 = mybir.dt.bfloat16
F32 = mybir.dt.float32
AF = mybir.ActivationFunctionType
OP = mybir.AluOpType

NP_BF16 = mybir.dt.np(BF16)


# ----------------------------------------------------------------------------
# Device program (SPMD; one core's shard)
# ----------------------------------------------------------------------------

def build_program(has_bias):
    nc = bacc.Bacc("TRN2", target_bir_lowering=False, debug=False,
                   num_devices=N_CORES)

    G = G_PER_CORE

    # inputs: pre-tiled on host so every DMA is contiguous per partition.
    # xa packs the per-graph X^T chunks (k=0..5) and An^T blocks (k=6,7)
    # into one stream so each graph needs a single 2KB-per-partition DMA.
    xa = nc.dram_tensor("xa", [128, G, KCH + 2, 128], BF16,
                        kind="ExternalInput").ap()
    w1p = nc.dram_tensor("w1p", [128, KCH, 2 * H_FEATS], BF16,
                         kind="ExternalInput").ap()
    w2h = nc.dram_tensor("w2h", [128, 2, 2, H_FEATS], BF16,
                         kind="ExternalInput").ap()     # [p, b, hi_chunk, ho]
    rvec = nc.dram_tensor("rvec", [G, 2 * H_FEATS], BF16,
                          kind="ExternalInput").ap()
    if has_bias:
        # [p, j] with j=(2b+c): b1 for layer1, b2 (pre-scaled by 1/128) layer2
        b1c = nc.dram_tensor("b1c", [128, 4], F32, kind="ExternalInput").ap()
        b2c = nc.dram_tensor("b2c", [128, 4], F32, kind="ExternalInput").ap()
    out = nc.dram_tensor("out", [G, 4 * H_FEATS], F32,
                         kind="ExternalOutput").ap()

    with tile.TileContext(nc) as tc:
        with (
            tc.tile_pool(name="const", bufs=1) as const,
            tc.tile_pool(name="xin", bufs=5) as xin,
            tc.tile_pool(name="atin", bufs=8) as atin,
            tc.tile_pool(name="ysb", bufs=2) as ysbp,
            tc.tile_pool(name="hsb", bufs=3) as hsbp,
            tc.tile_pool(name="zsb", bufs=3) as zsbp,
            tc.tile_pool(name="psY", bufs=2, space="PSUM") as psY,
            tc.tile_pool(name="psA", bufs=2, space="PSUM") as psA,
            tc.tile_pool(name="psZ", bufs=2, space="PSUM") as psZ,
            tc.tile_pool(name="psH", bufs=2, space="PSUM") as psH,
        ):
            # ---- input streaming -------------------------------------------
            # Issue order matters: the sync queue carries only what gates
            # Y(0); everything else goes to the gpsimd/scalar queues so the
            # critical stream gets the DMA bandwidth first.
            xa_t = {}

            def load_xa(i, eng):
                xa_t[i] = xin.tile([128, KCH + 2, 128], BF16, tag="xa",
                                   bufs=8, name=f"xa{i}")
                eng.dma_start(xa_t[i][:], xa[:, i, :, :])

            # graph 0 load split in two so Y(0) k=0..3 can start early
            xa_t[0] = xin.tile([128, KCH + 2, 128], BF16, tag="xa", bufs=8,
                               name="xa0")
            nc.sync.dma_start(xa_t[0][:, 0:4, :], xa[:, 0, 0:4, :])
            w1_sb = []
            for k in range(KCH):
                t = const.tile([128, 2 * H_FEATS], BF16, tag=f"w1_{k}",
                               name=f"w1_{k}")
                nc.sync.dma_start(t[:], w1p[:, k, :])
                w1_sb.append(t)
            nc.sync.dma_start(xa_t[0][:, 4:KCH + 2, :], xa[:, 0, 4:KCH + 2, :])
            w2h_sb = const.tile([128, 2, 2, H_FEATS], BF16)
            nc.scalar.dma_start(w2h_sb[:], w2h)
            load_xa(1, nc.scalar)
            identity_f32 = const.tile([128, 128], F32)
            make_identity(nc, identity_f32[:])
            if has_bias:
                b1_sb = const.tile([128, 4], F32)
                nc.scalar.dma_start(b1_sb[:], b1c)
                b2_sb = const.tile([128, 4], F32)
                nc.scalar.dma_start(b2_sb[:], b2c)

            # readout collect tiles ([p, g, j], j = 2b+c) -- g-major so the
            # transposed tile DMAs to out in one shot per half
            roots_sb = const.tile([128, G, 4], F32)
            means_sb = const.tile([128, G, 4], F32)

            # host-computed rvec, streamed per graph as a partition-broadcast
            # DMA ([1, 512] -> [128, 512]) so the root-input term of layer 2
            # is a pure DVE add (no PE rank-1 matmuls)
            rv_t = {}

            def load_rv(i, eng):
                rv_t[i] = atin.tile([128, 2 * H_FEATS], BF16, tag="rv",
                                    bufs=4, name=f"rv{i}")
                eng.dma_start(rv_t[i][:],
                              rvec[i:i + 1, :].partition_broadcast(128))

            # ---- software-pipelined main loop ------------------------------
            # Per-iteration PE emission interleaves the long Y matmuls
            # (N=512) between the short agg matmuls (N=128) so the next
            # LDWEIGHTS always has a long stream to hide behind.
            y_sb, h_sb, z_sb = {}, {}, {}
            ps_y = ps_a = None

            def y_mm(i, k):
                nc.tensor.matmul(ps_y[:], xa_t[i][:, k, :], w1_sb[k][:],
                                 start=(k == 0), stop=(k == KCH - 1))

            def agg1_mm(i, j):
                b, c = divmod(j, 2)
                nc.tensor.matmul(
                    ps_a[:, j * 128:(j + 1) * 128],
                    y_sb[i][:, b * 256 + c * 128: b * 256 + (c + 1) * 128],
                    xa_t[i][:, KCH + b, :])

            def evict_y(i):
                y_sb[i] = ysbp.tile([128, 2 * H_FEATS], BF16, tag="y",
                                    name=f"y{i}")
                nc.vector.tensor_copy(y_sb[i][:], ps_y[:])

            def evict_h(i):
                h_sb[i] = hsbp.tile([128, 4, 128], BF16, tag="h", name=f"h{i}")
                if has_bias:
                    for j in range(4):
                        nc.scalar.activation(h_sb[i][:, j, :],
                                             ps_a[:, j * 128:(j + 1) * 128],
                                             AF.Relu, bias=b1_sb[:, j:j + 1])
                else:
                    nc.scalar.activation(h_sb[i].rearrange("p a b -> p (a b)"),
                                         ps_a[:], AF.Relu)
                # collect h at root (node 0) for the readout
                nc.gpsimd.tensor_copy(roots_sb[:, i, :],
                                      h_sb[i][:, :, 0])
                del y_sb[i]

            def stage_z(i):
                ps_z = psZ.tile([128, 512], F32, tag="z", name=f"psz{i}")
                for b in (0, 1):
                    cols = slice(b * 256, (b + 1) * 256)
                    for c in (0, 1):
                        nc.tensor.matmul(ps_z[:, cols],
                                         h_sb[i][:, 2 * b + c, :],
                                         w2h_sb[:, b, c, :],
                                         start=(c == 0), stop=(c == 1))
                del h_sb[i]
                # evict halves on DVE+Scalar, then add rvec on gpsimd (Pool
                # cannot read PSUM, but the add is pure SBUF work)
                zt = zsbp.tile([128, 512], BF16, tag="ztmp", name=f"zt{i}")
                nc.vector.tensor_copy(zt[:, 0:256], ps_z[:, 0:256])
                nc.scalar.copy(zt[:, 256:512], ps_z[:, 256:512])
                z_sb[i] = zsbp.tile([128, 512], BF16, tag="z", name=f"z{i}")
                nc.gpsimd.tensor_tensor(z_sb[i][:], zt[:], rv_t[i][:],
                                        op=OP.add)
                del rv_t[i]

            def stage_agg2(i):
                ps_h = psH.tile([128, 512], F32, tag="h2", name=f"psh{i}")
                for b in (0, 1):
                    for c in (0, 1):
                        j = 2 * b + c
                        nc.tensor.matmul(
                            ps_h[:, j * 128:(j + 1) * 128],
                            z_sb[i][:, b * 256 + c * 128: b * 256 + (c + 1) * 128],
                            xa_t[i][:, KCH + b, :])
                del z_sb[i], xa_t[i]
                # evict with the mean's 1/128 pre-folded (relu(x/128) ==
                # relu(x)/128), then mean = plain row-sum on the DVE
                h2 = hsbp.tile([128, 4, 128], BF16, tag="h2", name=f"h2{i}")
                if has_bias:
                    for j in range(4):
                        nc.scalar.activation(h2[:, j, :],
                                             ps_h[:, j * 128:(j + 1) * 128],
                                             AF.Relu, bias=b2_sb[:, j:j + 1],
                                             scale=1.0 / N_PER_G)
                else:
                    nc.scalar.activation(h2.rearrange("p a b -> p (a b)"),
                                         ps_h[:], AF.Relu, scale=1.0 / N_PER_G)
                nc.vector.tensor_reduce(means_sb[:, i, :], h2[:],
                                        axis=mybir.AxisListType.X,
                                        op=OP.add)

            for i in range(G + 3):
                if i + 2 <= G - 1:
                    load_xa(i + 2, nc.sync)
                if i <= G - 1:
                    load_rv(i, nc.sync)
                do_y = i <= G - 1
                do_a1 = 1 <= i <= G
                if do_y:
                    ps_y = psY.tile([128, 2 * H_FEATS], F32, tag="y",
                                    name=f"psy{i}")
                if do_a1:
                    ps_a = psA.tile([128, 512], F32, tag="a", name=f"psa{i}")
                # interleaved PE emission: Y(i) x6 between agg1(i-1) x4
                for k in range(KCH):
                    if do_y:
                        y_mm(i, k)
                    if do_a1 and k < 4:
                        agg1_mm(i - 1, k)
                if do_y:
                    evict_y(i)
                if do_a1:
                    evict_h(i - 1)
                if 2 <= i <= G + 1:
                    stage_z(i - 2)
                if 3 <= i <= G + 2:
                    stage_agg2(i - 3)

            # ---- readout: transpose collect tiles, write out ---------------
            # transposed partition index is (g, b, c); the rearranged dram
            # view walks (g, b, c, p) in the same order -> one DMA per half
            out5 = out.rearrange("g (b h c p) -> g b h c p", b=2, h=2, c=2,
                                 p=128)
            out_eng = [nc.sync, nc.scalar]
            for idx, (src_t, half) in enumerate(((means_sb, 0),
                                                 (roots_sb, 1))):
                ps_t = psZ.tile([128, 128], F32, tag="z", name=f"tr{idx}")
                nc.tensor.transpose(ps_t[:],
                                    src_t.rearrange("p g a -> p (g a)"),
                                    identity_f32[:])
                ot = hsbp.tile([128, 128], F32, tag="otile", name=f"ot{idx}")
                nc.scalar.copy(ot[:], ps_t[:])
                out_eng[idx].dma_start(out5[:, :, half, :, :], ot[:])

    nc.compile()
    return nc


# ----------------------------------------------------------------------------
# Host-side packing
# ----------------------------------------------------------------------------

def _build_adj(src, dst, n, G):
    """Dense normalized-adjacency-transpose blocks An^T per graph, f32.
    An = D^-1/2 A D^-1/2 with A[d, s] = #edges s->d (self-loops included in
    the given edge lists).  Returns [G, n, n] with block[g, s, d]."""
    src = np.asarray(src, np.int64)
    dst = np.asarray(dst, np.int64)
    N = n * G
    g = dst // n
    if not np.array_equal(src // n, g):
        raise ValueError("cross-graph edge found; contiguous-block sharding invalid")
    deg = np.bincount(dst, minlength=N).astype(np.float32)
    norm = 1.0 / np.sqrt(np.maximum(deg, 1e-30))
    w = (norm[src] * norm[dst]).astype(np.float32)
    at = np.zeros((G, n, n), np.float32)
    np.add.at(at, (g, src - g * n, dst - g * n), w)
    return at


def _prep(inputs, w1_td, b1_td, w2_td, b2_td, w1_bu, b1_bu, w2_bu, b2_bu,
          td_src, td_dst, bu_src, bu_dst, nodes_per_graph):
    n = int(nodes_per_graph)
    X = np.asarray(inputs, np.float32)
    N = X.shape[0]
    G = N // n
    assert (n, G, X.shape[1]) == (N_PER_G, N_GRAPHS, IN_FEATS), \
        f"unexpected shapes {X.shape} n={n}"

    at_td = _build_adj(td_src, td_dst, n, G)
    at_bu = _build_adj(bu_src, bu_dst, n, G)
    # [G, 2, n_s, n_d]
    at_all = np.stack([at_td, at_bu], axis=1).astype(NP_BF16)

    w1p = np.concatenate([np.asarray(w1_td, np.float32),
                          np.asarray(w1_bu, np.float32)], axis=1)
    w1p = np.ascontiguousarray(
        w1p.reshape(KCH, 128, 2 * H_FEATS).transpose(1, 0, 2)).astype(NP_BF16)
    w2_td = np.asarray(w2_td, np.float32)
    w2_bu = np.asarray(w2_bu, np.float32)
    # [p, b, hi_chunk, ho]
    w2h = np.stack([w2_td[:H_FEATS].reshape(2, 128, H_FEATS),
                    w2_bu[:H_FEATS].reshape(2, 128, H_FEATS)], axis=0)
    w2h = np.ascontiguousarray(w2h.transpose(2, 0, 1, 3)).astype(NP_BF16)
    w2rp = np.concatenate([w2_td[H_FEATS:], w2_bu[H_FEATS:]], axis=1)
    rvec_all = (X[::n].astype(np.float32) @ w2rp).astype(NP_BF16)  # [G, 512]

    biases = [np.asarray(b, np.float32) for b in (b1_td, b2_td, b1_bu, b2_bu)]
    has_bias = any(np.any(b != 0) for b in biases)
    if has_bias:
        b1c = np.stack([biases[0][:128], biases[0][128:],
                        biases[2][:128], biases[2][128:]], axis=1)
        b2c = np.stack([biases[1][:128], biases[1][128:],
                        biases[3][:128], biases[3][128:]], axis=1) / N_PER_G
        b1c = np.ascontiguousarray(b1c, np.float32)
        b2c = np.ascontiguousarray(b2c, np.float32)

    in_maps = []
    for c in range(N_CORES):
        gs = slice(c * G_PER_CORE, (c + 1) * G_PER_CORE)
        ns = slice(c * NODES_PER_CORE, (c + 1) * NODES_PER_CORE)
        Xc = X[ns]
        # xa[p, g, 0:6, n] = X[g*128 + n, k*128 + p]; xa[p, g, 6+b, d] = AnT
        xt = Xc.reshape(G_PER_CORE, 128, KCH, 128).transpose(3, 0, 2, 1)
        xa = np.concatenate(
            [xt.astype(NP_BF16), at_all[gs].transpose(2, 0, 1, 3)], axis=2)
        m = {
            "xa": np.ascontiguousarray(xa),
            "w1p": w1p,
            "w2h": w2h,
            "rvec": np.ascontiguousarray(rvec_all[gs]),
        }
        if has_bias:
            m["b1c"] = b1c
            m["b2c"] = b2c
        in_maps.append(m)
    return in_maps, (has_bias,)


_PROGRAM_CACHE = {}


def _get_program(key):
    if key not in _PROGRAM_CACHE:
        _PROGRAM_CACHE[key] = build_program(*key)
    return _PROGRAM_CACHE[key]


def kernel(trace=False, tmpdir=None, _return_raw=False, **inputs):
    in_maps, key = _prep(**inputs)
    nc = _get_program(key)
    res = run_bass_kernel_spmd(nc, in_maps, list(range(N_CORES)),
                               trace=trace, tmpdir=tmpdir)
    out = np.concatenate([res.results[i]["out"] for i in range(N_CORES)], axis=0)
    if _return_raw:
        return out, res
    return out


# revision 3
# speedup vs baseline: 1.0033x; 1.0033x over previous
"""BiGCN (bidirectional 2-layer GCN over many small graphs) on 8 Trainium2 cores.

Data-parallel over graphs; 32 graphs x 128 nodes per core.

The GCN norms (D^-1/2 A D^-1/2, self-loops included) depend only on the edge
lists, so the normalized adjacency An^T for every (graph, branch) is built on
the HOST as a dense bf16 [128, 128] block and DMA'd in, along with its row
sums dt = An @ 1.  On-device work is a pure dense-matmul chain with
alternating layouts so no PE transposes are needed:

  y   = X @ [W1_td | W1_bu]                  N-layout [node, 512]     (PE)
  hT  = relu((An @ y)^T) per branch/chunk    H-layout [h, node]       (PE+Act)
  Z   = h @ W2h                              N-layout [node, 256]     (PE)
  H2T = relu((An @ Z)^T + rvec (x) dt)       H-layout [h, node]       (PE+Act)
  out = [mean_d H2, h[:, root]]              (reduce on DVE; transposed once)

rvec = X[roots] @ [W2r_td | W2r_bu] is computed once per core; the root-input
term of layer 2 folds into agg2 as the rank-1 update rvec (x) dt (K=1 matmul).

The per-graph loop is software-pipelined -- iteration i emits
Y(i) | agg1(i-1) | Z(i-2) | agg2(i-3), with Y's six 512-wide matmuls
interleaved between the short agg matmuls so LDWEIGHTS stays hidden and the
PE never stalls (keeping it at the 2.4GHz p-state).
"""

import numpy as np

import concourse.bass as bass
import concourse.tile as tile
from concourse import bacc, mybir
from concourse.bass_utils import run_bass_kernel_spmd
from concourse.masks import make_identity

# Problem shape (fixed by the task)
N_GRAPHS = 256
N_PER_G = 128
IN_FEATS = 768
H_FEATS = 256
N_CORES = 8
G_PER_CORE = N_GRAPHS // N_CORES            # 32
NODES_PER_CORE = G_PER_CORE * N_PER_G       # 4096
KCH = IN_FEATS // 128                       # 6 feature chunks

BF16 = mybir.dt.bfloat16
F32 = mybir.dt.float32
AF = mybir.ActivationFunctionType
OP = mybir.AluOpType

NP_BF16 = mybir.dt.np(BF16)


# ----------------------------------------------------------------------------
# Device program (SPMD; one core's shard)
# ----------------------------------------------------------------------------

def build_program(has_bias):
    nc = bacc.Bacc("TRN2", target_bir_lowering=False, debug=False,
                   num_devices=N_CORES)

    G = G_PER_CORE

    # inputs: pre-tiled on host so every DMA is contiguous per partition.
    # xa packs the per-graph X^T chunks (k=0..5) and An^T blocks (k=6,7)
    # into one stream so each graph needs a single 2KB-per-partition DMA.
    xa = nc.dram_tensor("xa", [128, G, KCH + 2, 128], BF16,
                        kind="ExternalInput").ap()
    w1p = nc.dram_tensor("w1p", [128, KCH, 2 * H_FEATS], BF16,
                         kind="ExternalInput").ap()
    w2h = nc.dram_tensor("w2h", [128, 2, 2, H_FEATS], BF16,
                         kind="ExternalInput").ap()     # [p, b, hi_chunk, ho]
    rvec = nc.dram_tensor("rvec", [G, 2 * H_FEATS], BF16,
                          kind="ExternalInput").ap()
    if has_bias:
        # [p, j] with j=(2b+c): b1 for layer1, b2 (pre-scaled by 1/128) layer2
        b1c = nc.dram_tensor("b1c", [128, 4], F32, kind="ExternalInput").ap()
        b2c = nc.dram_tensor("b2c", [128, 4], F32, kind="ExternalInput").ap()
    out = nc.dram_tensor("out", [G, 4 * H_FEATS], F32,
                         kind="ExternalOutput").ap()

    with tile.TileContext(nc) as tc:
        with (
            tc.tile_pool(name="const", bufs=1) as const,
            tc.tile_pool(name="xin", bufs=5) as xin,
            tc.tile_pool(name="atin", bufs=8) as atin,
            tc.tile_pool(name="ysb", bufs=2) as ysbp,
            tc.tile_pool(name="hsb", bufs=3) as hsbp,
            tc.tile_pool(name="zsb", bufs=3) as zsbp,
            tc.tile_pool(name="psY", bufs=2, space="PSUM") as psY,
            tc.tile_pool(name="psA", bufs=2, space="PSUM") as psA,
            tc.tile_pool(name="psZ", bufs=2, space="PSUM") as psZ,
            tc.tile_pool(name="psH", bufs=2, space="PSUM") as psH,
        ):
            # ---- input streaming -------------------------------------------
            # Issue order matters: the sync queue carries only what gates
            # Y(0); everything else goes to the gpsimd/scalar queues so the
            # critical stream gets the DMA bandwidth first.
            xa_t = {}

            def load_xa(i, eng):
                xa_t[i] = xin.tile([128, KCH + 2, 128], BF16, tag="xa",
                                   bufs=8, name=f"xa{i}")
                eng.dma_start(xa_t[i][:], xa[:, i, :, :])

            # graph 0 load split in two so Y(0) k=0..3 can start early
            xa_t[0] = xin.tile([128, KCH + 2, 128], BF16, tag="xa", bufs=8,
                               name="xa0")
            nc.sync.dma_start(xa_t[0][:, 0:4, :], xa[:, 0, 0:4, :])
            w1_sb = []
            for k in range(KCH):
                t = const.tile([128, 2 * H_FEATS], BF16, tag=f"w1_{k}",
                               name=f"w1_{k}")
                nc.sync.dma_start(t[:], w1p[:, k, :])
                w1_sb.append(t)
            nc.sync.dma_start(xa_t[0][:, 4:KCH + 2, :], xa[:, 0, 4:KCH + 2, :])
            w2h_sb = const.tile([128, 2, 2, H_FEATS], BF16)
            nc.scalar.dma_start(w2h_sb[:], w2h)
            load_xa(1, nc.scalar)
            rvec_row = const.tile([1, G * 2 * H_FEATS], BF16)
            nc.scalar.dma_start(rvec_row[:],
                                rvec.rearrange("g f -> (g f)")[None, :])
            ones_row_f32 = const.tile([1, 128], F32)
            nc.gpsimd.memset(ones_row_f32[:], 1.0)
            ones_row = const.tile([1, 128], BF16)
            nc.vector.tensor_copy(ones_row[:], ones_row_f32[:])
            identity_f32 = const.tile([128, 128], F32)
            make_identity(nc, identity_f32[:])
            if has_bias:
                b1_sb = const.tile([128, 4], F32)
                nc.scalar.dma_start(b1_sb[:], b1c)
                b2_sb = const.tile([128, 4], F32)
                nc.scalar.dma_start(b2_sb[:], b2c)

            # readout collect tiles ([p, g, j], j = 2b+c) -- g-major so the
            # transposed tile DMAs to out in one shot per half
            roots_sb = const.tile([128, G, 4], F32)
            means_sb = const.tile([128, G, 4], F32)

            # host-computed rvec, streamed per graph as a partition-broadcast
            # DMA ([1, 512] -> [128, 512]) so the root-input term of layer 2
            # is a pure DVE add (no PE rank-1 matmuls)
            rv_t = {}

            def load_rv(i, eng):
                rv_t[i] = atin.tile([128, 2 * H_FEATS], BF16, tag="rv",
                                    bufs=4, name=f"rv{i}")
                eng.dma_start(rv_t[i][:],
                              rvec[i:i + 1, :].partition_broadcast(128))

            # ---- software-pipelined main loop ------------------------------
            # Per-iteration PE emission interleaves the long Y matmuls
            # (N=512) between the short agg matmuls (N=128) so the next
            # LDWEIGHTS always has a long stream to hide behind.
            y_sb, h_sb, z_sb = {}, {}, {}
            ps_y = ps_a = None

            def y_mm(i, k):
                nc.tensor.matmul(ps_y[:], xa_t[i][:, k, :], w1_sb[k][:],
                                 start=(k == 0), stop=(k == KCH - 1))

            def agg1_mm(i, j):
                b, c = divmod(j, 2)
                nc.tensor.matmul(
                    ps_a[:, j * 128:(j + 1) * 128],
                    y_sb[i][:, b * 256 + c * 128: b * 256 + (c + 1) * 128],
                    xa_t[i][:, KCH + b, :])

            def evict_y(i):
                y_sb[i] = ysbp.tile([128, 2 * H_FEATS], BF16, tag="y",
                                    name=f"y{i}")
                nc.vector.tensor_copy(y_sb[i][:], ps_y[:])

            def evict_h(i):
                h_sb[i] = hsbp.tile([128, 4, 128], BF16, tag="h", name=f"h{i}")
                if has_bias:
                    for j in range(4):
                        nc.scalar.activation(h_sb[i][:, j, :],
                                             ps_a[:, j * 128:(j + 1) * 128],
                                             AF.Relu, bias=b1_sb[:, j:j + 1])
                else:
                    nc.scalar.activation(h_sb[i].rearrange("p a b -> p (a b)"),
                                         ps_a[:], AF.Relu)
                # collect h at root (node 0) for the readout
                nc.gpsimd.tensor_copy(roots_sb[:, i, :],
                                      h_sb[i][:, :, 0])
                del y_sb[i]

            def stage_z(i):
                tail = i >= G - 2
                ps_z = psZ.tile([128, 512], F32, tag="z", name=f"psz{i}")
                for b in (0, 1):
                    cols = slice(b * 256, (b + 1) * 256)
                    for c in (0, 1):
                        nc.tensor.matmul(ps_z[:, cols],
                                         h_sb[i][:, 2 * b + c, :],
                                         w2h_sb[:, b, c, :],
                                         start=(c == 0),
                                         stop=(c == 1 and not tail))
                    if tail:
                        # drain path: fold rvec in-PSUM (PE is idle here and
                        # this avoids the 3-engine evict chain)
                        off = i * 512 + b * 256
                        nc.tensor.matmul(ps_z[:, cols], ones_row[:],
                                         rvec_row[0:1, off:off + 256],
                                         start=False, stop=True)
                z_sb[i] = zsbp.tile([128, 512], BF16, tag="z", name=f"z{i}")
                if tail:
                    nc.scalar.copy(z_sb[i][:], ps_z[:])
                else:
                    # evict halves on DVE+Scalar, then add rvec on gpsimd
                    # (Pool cannot read PSUM, but the add is pure SBUF work)
                    zt = zsbp.tile([128, 512], BF16, tag="ztmp", name=f"zt{i}")
                    nc.vector.tensor_copy(zt[:, 0:256], ps_z[:, 0:256])
                    nc.scalar.copy(zt[:, 256:512], ps_z[:, 256:512])
                    nc.gpsimd.tensor_tensor(z_sb[i][:], zt[:], rv_t[i][:],
                                            op=OP.add)
                    del rv_t[i]
                del h_sb[i]

            def stage_agg2(i):
                ps_h = psH.tile([128, 512], F32, tag="h2", name=f"psh{i}")
                for b in (0, 1):
                    for c in (0, 1):
                        j = 2 * b + c
                        nc.tensor.matmul(
                            ps_h[:, j * 128:(j + 1) * 128],
                            z_sb[i][:, b * 256 + c * 128: b * 256 + (c + 1) * 128],
                            xa_t[i][:, KCH + b, :])
                del z_sb[i], xa_t[i]
                # evict with the mean's 1/128 pre-folded (relu(x/128) ==
                # relu(x)/128), then mean = plain row-sum on the DVE
                h2 = hsbp.tile([128, 4, 128], BF16, tag="h2", name=f"h2{i}")
                if has_bias:
                    for j in range(4):
                        nc.scalar.activation(h2[:, j, :],
                                             ps_h[:, j * 128:(j + 1) * 128],
                                             AF.Relu, bias=b2_sb[:, j:j + 1],
                                             scale=1.0 / N_PER_G)
                else:
                    nc.scalar.activation(h2.rearrange("p a b -> p (a b)"),
                                         ps_h[:], AF.Relu, scale=1.0 / N_PER_G)
                nc.vector.tensor_reduce(means_sb[:, i, :], h2[:],
                                        axis=mybir.AxisListType.X,
                                        op=OP.add)

            # readout: transposed partition index is (g, b, c); the
            # rearranged dram view walks (g, b, c, p) in the same order ->
            # one DMA per (half, graph-range)
            out5 = out.rearrange("g (b h c p) -> g b h c p", b=2, h=2, c=2,
                                 p=128)
            out_eng = [nc.sync, nc.scalar]

            def emit_readout(g0, g1):
                for idx, (src_t, half) in enumerate(((means_sb, 0),
                                                     (roots_sb, 1))):
                    n = (g1 - g0) * 4
                    ps_t = psZ.tile([128, 128], F32, tag="z",
                                    name=f"tr{half}_{g0}")
                    nc.tensor.transpose(
                        ps_t[0:n, :],
                        src_t[:, g0:g1, :].rearrange("p g a -> p (g a)"),
                        identity_f32[:])
                    ot = hsbp.tile([128, 128], F32, tag="otile",
                                   name=f"ot{half}_{g0}")
                    nc.scalar.copy(ot[0:n, :], ps_t[0:n, :])
                    out_eng[idx].dma_start(out5[g0:g1, :, half, :, :],
                                           ot[0:n, :])

            for i in range(G + 3):
                if i + 2 <= G - 1:
                    load_xa(i + 2, nc.sync)
                if i <= G - 3:
                    load_rv(i, nc.sync)
                do_y = i <= G - 1
                do_a1 = 1 <= i <= G
                if do_y:
                    ps_y = psY.tile([128, 2 * H_FEATS], F32, tag="y",
                                    name=f"psy{i}")
                if do_a1:
                    ps_a = psA.tile([128, 512], F32, tag="a", name=f"psa{i}")
                # interleaved PE emission: Y(i) x6 between agg1(i-1) x4
                for k in range(KCH):
                    if do_y:
                        y_mm(i, k)
                    if do_a1 and k < 4:
                        agg1_mm(i - 1, k)
                if do_y:
                    evict_y(i)
                if do_a1:
                    evict_h(i - 1)
                if 2 <= i <= G + 1:
                    stage_z(i - 2)
                if 3 <= i <= G + 2:
                    stage_agg2(i - 3)
                if i == G - 9:
                    emit_readout(0, G - 12)

            emit_readout(G - 12, G)


    nc.compile()
    return nc


# ----------------------------------------------------------------------------
# Host-side packing
# ----------------------------------------------------------------------------

def _build_adj(src, dst, n, G):
    """Dense normalized-adjacency-transpose blocks An^T per graph, f32.
    An = D^-1/2 A D^-1/2 with A[d, s] = #edges s->d (self-loops included in
    the given edge lists).  Returns [G, n, n] with block[g, s, d]."""
    src = np.asarray(src, np.int64)
    dst = np.asarray(dst, np.int64)
    N = n * G
    g = dst // n
    if not np.array_equal(src // n, g):
        raise ValueError("cross-graph edge found; contiguous-block sharding invalid")
    deg = np.bincount(dst, minlength=N).astype(np.float32)
    norm = 1.0 / np.sqrt(np.maximum(deg, 1e-30))
    w = (norm[src] * norm[dst]).astype(np.float32)
    at = np.zeros((G, n, n), np.float32)
    np.add.at(at, (g, src - g * n, dst - g * n), w)
    return at


def _prep(inputs, w1_td, b1_td, w2_td, b2_td, w1_bu, b1_bu, w2_bu, b2_bu,
          td_src, td_dst, bu_src, bu_dst, nodes_per_graph):
    n = int(nodes_per_graph)
    X = np.asarray(inputs, np.float32)
    N = X.shape[0]
    G = N // n
    assert (n, G, X.shape[1]) == (N_PER_G, N_GRAPHS, IN_FEATS), \
        f"unexpected shapes {X.shape} n={n}"

    at_td = _build_adj(td_src, td_dst, n, G)
    at_bu = _build_adj(bu_src, bu_dst, n, G)
    # [G, 2, n_s, n_d]
    at_all = np.stack([at_td, at_bu], axis=1).astype(NP_BF16)

    w1p = np.concatenate([np.asarray(w1_td, np.float32),
                          np.asarray(w1_bu, np.float32)], axis=1)
    w1p = np.ascontiguousarray(
        w1p.reshape(KCH, 128, 2 * H_FEATS).transpose(1, 0, 2)).astype(NP_BF16)
    w2_td = np.asarray(w2_td, np.float32)
    w2_bu = np.asarray(w2_bu, np.float32)
    # [p, b, hi_chunk, ho]
    w2h = np.stack([w2_td[:H_FEATS].reshape(2, 128, H_FEATS),
                    w2_bu[:H_FEATS].reshape(2, 128, H_FEATS)], axis=0)
    w2h = np.ascontiguousarray(w2h.transpose(2, 0, 1, 3)).astype(NP_BF16)
    w2rp = np.concatenate([w2_td[H_FEATS:], w2_bu[H_FEATS:]], axis=1)
    rvec_all = (X[::n].astype(np.float32) @ w2rp).astype(NP_BF16)  # [G, 512]

    biases = [np.asarray(b, np.float32) for b in (b1_td, b2_td, b1_bu, b2_bu)]
    has_bias = any(np.any(b != 0) for b in biases)
    if has_bias:
        b1c = np.stack([biases[0][:128], biases[0][128:],
                        biases[2][:128], biases[2][128:]], axis=1)
        b2c = np.stack([biases[1][:128], biases[1][128:],
                        biases[3][:128], biases[3][128:]], axis=1) / N_PER_G
        b1c = np.ascontiguousarray(b1c, np.float32)
        b2c = np.ascontiguousarray(b2c, np.float32)

    in_maps = []
    for c in range(N_CORES):
        gs = slice(c * G_PER_CORE, (c + 1) * G_PER_CORE)
        ns = slice(c * NODES_PER_CORE, (c + 1) * NODES_PER_CORE)
        Xc = X[ns]
        # xa[p, g, 0:6, n] = X[g*128 + n, k*128 + p]; xa[p, g, 6+b, d] = AnT
        xt = Xc.reshape(G_PER_CORE, 128, KCH, 128).transpose(3, 0, 2, 1)
        xa = np.concatenate(
            [xt.astype(NP_BF16), at_all[gs].transpose(2, 0, 1, 3)], axis=2)
        m = {
            "xa": np.ascontiguousarray(xa),
            "w1p": w1p,
            "w2h": w2h,
            "rvec": np.ascontiguousarray(rvec_all[gs]),
        }
        if has_bias:
            m["b1c"] = b1c
            m["b2c"] = b2c
        in_maps.append(m)
    return in_maps, (has_bias,)


_PROGRAM_CACHE = {}


def _get_program(key):
    if key not in _PROGRAM_CACHE:
        _PROGRAM_CACHE[key] = build_program(*key)
    return _PROGRAM_CACHE[key]


def kernel(trace=False, tmpdir=None, _return_raw=False, **inputs):
    in_maps, key = _prep(**inputs)
    nc = _get_program(key)
    res = run_bass_kernel_spmd(nc, in_maps, list(range(N_CORES)),
                               trace=trace, tmpdir=tmpdir)
    out = np.concatenate([res.results[i]["out"] for i in range(N_CORES)], axis=0)
    if _return_raw:
        return out, res
    return out
